# revision 1
# baseline (speedup 1.0000x reference)
"""AttentionLePE distributed Trainium2 kernel.

Strategy: pure data-parallel over batch (8 batch elements -> 8 NeuronCores,
no collectives). Per core, the full attention block runs with:
  - bf16 matmuls on TensorE (qkv, s^T = k q^T with 4-head row-packing,
    p@v + all-ones denominator matmuls with 4-head col-packing, proj)
  - softmax exp on ScalarE straight out of PSUM (no max-subtraction: logits
    are ~N(0,1) so exp is safe in f32)
  - LePE 5x5 depthwise conv split across engines: the 3x3 inner taps as
    accumulating diagonal matmuls on TensorE with spatially shifted access
    patterns (zero-pad handled by trimming), the 16 outer-ring taps as fused
    affine MACs (affine_then_add) on VectorE
  - normalization via all-ones matmul row sums (replicated to each head's 32
    output rows) -> fast reciprocal on VectorE straight from PSUM -> fused
    multiply during PSUM evacuation

The whole kernel is software-pipelined at emission time as one flat 64-step
sweep sequence: each engine's in-order stream gets p@v of step m-1 plus
deadline-scheduled filler (v/qk tiles, LePE taps, early proj halves) between
s^T(m) and s^T(m+1), so TensorE stays busy while ScalarE runs exp(m) and
exp never stalls at sweep boundaries.

Host side pre-transposes x and all weights so no on-device transposes are
needed, and folds b_lepe through w_proj into an effective bias.
"""

import numpy as np
import ml_dtypes

B, Hs, Ws, C = 8, 32, 32, 512
N = Hs * Ws          # 1024 tokens
HEADS = 16
HD = C // HEADS      # 32
KS = 5
SCALE = float(HD) ** -0.5
NCORES = 8

_BF16 = ml_dtypes.bfloat16

LAST_EXEC_TIME_NS = None
LAST_RESULTS = None


def _build_graph():
    import os as _os
    LOOP = int(_os.environ.get("ATTN_LEPE_LOOP", "1"))
    DBG = _os.environ.get("ATTN_LEPE_DEBUG", "") == "1"
    import concourse.bacc as bacc
    import concourse.mybir as mybir
    import concourse.tile as tile

    dt = mybir.dt
    AF = mybir.ActivationFunctionType

    nc = bacc.Bacc(
        "TRN2",
        target_bir_lowering=False,
        debug=False,
        enable_asserts=False,
        num_devices=NCORES,
    )

    xT_d = nc.dram_tensor("xT", [C, N], dt.bfloat16, kind="ExternalInput")
    wqkT_d = nc.dram_tensor("wqkT", [C, 2 * C], dt.bfloat16, kind="ExternalInput")
    wvT_d = nc.dram_tensor("wvT", [C, C], dt.bfloat16, kind="ExternalInput")
    wpT_d = nc.dram_tensor("wpT", [C, C], dt.bfloat16, kind="ExternalInput")
    ones_d = nc.dram_tensor("ones", [128, 32], dt.bfloat16, kind="ExternalInput")
    # lepe_d[p, (pi*4+g)*128 + q] = (p==q) * w_lepe[128*g+p, inner tap pi]
    lepe_d = nc.dram_tensor("lepe", [128, 9 * 4 * 128], dt.bfloat16,
                            kind="ExternalInput")
    lepec_d = nc.dram_tensor("lepec", [128, KS * KS * 4], dt.float32,
                             kind="ExternalInput")
    beff_d = nc.dram_tensor("beff", [128, 4], dt.float32, kind="ExternalInput")
    out_d = nc.dram_tensor("out", [C, N], dt.float32, kind="ExternalOutput")
    if DBG:
        dbg_pT = nc.dram_tensor("dbg_pT", [128, 2048], dt.bfloat16,
                                kind="ExternalOutput")
        dbg_ao0 = nc.dram_tensor("dbg_ao0", [128, N], dt.bfloat16,
                                 kind="ExternalOutput")

    NT = N // 128   # 8 token tiles
    CT = C // 128   # 4 channel tiles
    NC2 = N // 512  # 2 n-chunks

    taps = [(0, 0)] + [
        (dh, dw) for dh in range(-2, 3) for dw in range(-2, 3) if (dh, dw) != (0, 0)
    ]

    with tile.TileContext(nc) as tc:
        with (
            tc.tile_pool(name="persist", bufs=1) as persist,
            tc.tile_pool(name="pT", bufs=4) as pT_pool,
            tc.tile_pool(name="dr", bufs=3) as dr_pool,
            tc.tile_pool(name="ps_big", bufs=1, space="PSUM") as ps_big,
            tc.tile_pool(name="ps_small", bufs=2, space="PSUM") as ps_small,
            tc.tile_pool(name="ps_den", bufs=1, space="PSUM") as ps_den,
            tc.tile_pool(name="ps_lepe", bufs=1, space="PSUM") as ps_lepe,
        ):
            # ---- persistent SBUF loads ----
            xT = []
            for g in range(CT):
                t = persist.tile([128, N], dt.bfloat16, tag=f"xT{g}", name=f"xT{g}")
                nc.sync.dma_start(t[:], xT_d[g * 128:(g + 1) * 128, :])
                xT.append(t)
            wqkT = []
            for g in range(CT):
                t = persist.tile([128, 2 * C], dt.bfloat16, tag=f"wqkT{g}",
                                 name=f"wqkT{g}")
                nc.sync.dma_start(t[:], wqkT_d[g * 128:(g + 1) * 128, :])
                wqkT.append(t)
            wvT = []
            for g in range(CT):
                t = persist.tile([128, C], dt.bfloat16, tag=f"wvT{g}", name=f"wvT{g}")
                nc.sync.dma_start(t[:], wvT_d[g * 128:(g + 1) * 128, :])
                wvT.append(t)
            ones_sb = persist.tile([128, 32], dt.bfloat16, tag="ones", name="ones_sb")
            nc.sync.dma_start(ones_sb[:], ones_d[:, :])
            # non-critical loads (LePE table, proj weights) are deferred past
            # the head kickoff so the x/w_qk/w_v preload gets full DMA
            # bandwidth
            wpT = [persist.tile([128, C], dt.bfloat16, tag=f"wpT{g}",
                                name=f"wpT{g}") for g in range(CT)]
            lepe_w = persist.tile([128, 9 * 4 * 128], dt.bfloat16, tag="lepe",
                                  name="lepe_w")
            beff_sb = persist.tile([128, 4], dt.float32, tag="beff", name="beff_sb")
            lepec_sb = persist.tile([128, KS * KS * 4], dt.float32, tag="lepec",
                                    name="lepec_sb")

            def load_noncritical():
                nc.sync.dma_start(lepe_w[:], lepe_d[:, :])
                nc.sync.dma_start(lepec_sb[:], lepec_d[:, :])
                for g in range(CT):
                    nc.sync.dma_start(wpT[g][:], wpT_d[g * 128:(g + 1) * 128, :])
                nc.sync.dma_start(beff_sb[:], beff_d[:, :])

            for _it in range(LOOP):
                # ---------- tiles ----------
                v_sb = [persist.tile([128, 512], dt.bfloat16, tag=f"v{m}",
                                     name=f"v{m}") for m in range(NT)]
                qk_sb = [
                    persist.tile([128, N], dt.bfloat16, tag=f"qk{f}", name=f"qk{f}")
                    for f in range(8)
                ]
                aoT = [persist.tile([128, N], dt.bfloat16, tag=f"aoT{g}",
                                    name=f"aoT{g}") for g in range(4)]
                yT_sb = [persist.tile([128, N], dt.float32, tag=f"yT{co}",
                                      name=f"yT{co}") for co in range(CT)]
                x3 = [xT[g][:].rearrange("p (h w) -> p h w", w=Ws)
                      for g in range(CT)]

                def emit_v(m):
                    v_ps = ps_small.tile([128, 512], dt.float32, tag="sm",
                                         name=f"vps{m}")
                    for c in range(CT):
                        nc.tensor.matmul(
                            out=v_ps[:],
                            lhsT=xT[c][:, m * 128:(m + 1) * 128],
                            rhs=wvT[c][:],
                            start=(c == 0), stop=(c == CT - 1),
                        )
                    nc.vector.tensor_copy(v_sb[m][:], v_ps[:])

                def emit_qk(f, nc2):
                    qk_ps = ps_small.tile([128, 512], dt.float32, tag="sm",
                                          name=f"qkps{f}_{nc2}")
                    for c in range(CT):
                        nc.tensor.matmul(
                            out=qk_ps[:],
                            lhsT=wqkT[c][:, f * 128:(f + 1) * 128],
                            rhs=xT[c][:, nc2 * 512:(nc2 + 1) * 512],
                            start=(c == 0), stop=(c == CT - 1),
                        )
                    nc.vector.tensor_copy(
                        qk_sb[f][:, nc2 * 512:(nc2 + 1) * 512], qk_ps[:])

                def emit_proj(co, nc2):
                    ncs = slice(nc2 * 512, (nc2 + 1) * 512)
                    y_ps = ps_small.tile([128, 512], dt.float32, tag="sm",
                                         name=f"yps{co}_{nc2}")
                    for c in range(CT):
                        nc.tensor.matmul(
                            out=y_ps[:],
                            lhsT=wpT[c][:, co * 128:(co + 1) * 128],
                            rhs=aoT[c][:, ncs],
                            start=(c == 0), stop=(c == CT - 1),
                        )
                    nc.vector.tensor_scalar_add(
                        yT_sb[co][:, ncs], y_ps[:], beff_sb[:, co:co + 1])
                    # ship each output half as soon as its proj is done
                    nc.sync.dma_start(out_d[co * 128:(co + 1) * 128, ncs],
                                      yT_sb[co][:, ncs])

                # 3x3 inner taps on TensorE (diag matmuls), 16 outer-ring
                # taps on VectorE (fused affine MAC)
                pe_taps = [(dh, dw) for (dh, dw) in taps
                           if abs(dh) <= 1 and abs(dw) <= 1]
                dve_taps = [t for t in taps if t not in pe_taps]

                def lepe_mms(g, hb, lp3):
                    mms = []
                    for pi, (dh, dw) in enumerate(pe_taps):
                        r0, r1 = max(0, -dh), Hs - max(0, dh)
                        w0, w1 = max(0, -dw), Ws - max(0, dw)
                        hr0, hr1 = max(r0, hb * 16), min(r1, hb * 16 + 16)
                        if hr0 >= hr1:
                            continue
                        diag = lepe_w[:, (pi * 4 + g) * 128:(pi * 4 + g + 1) * 128]

                        def mm(pi=pi, hr0=hr0, hr1=hr1, w0=w0, w1=w1,
                               dh=dh, dw=dw, diag=diag, lp3=lp3, g=g, hb=hb):
                            nc.tensor.matmul(
                                out=lp3[:, hr0 - hb * 16:hr1 - hb * 16, w0:w1],
                                lhsT=diag,
                                rhs=x3[g][:, hr0 + dh:hr1 + dh, w0 + dw:w1 + dw],
                                start=(pi == 0), stop=(pi == len(pe_taps) - 1),
                                skip_group_check=True,
                            )
                        mms.append(mm)
                    return mms

                def lepe_dve_units(g, hb, acc):
                    acc3 = acc[:].rearrange("p (h w) -> p h w", w=Ws)
                    units = []
                    for dh, dw in dve_taps:
                        ti = taps.index((dh, dw))
                        r0, r1 = max(0, -dh), Hs - max(0, dh)
                        w0, w1 = max(0, -dw), Ws - max(0, dw)
                        hr0, hr1 = max(r0, hb * 16), min(r1, hb * 16 + 16)
                        if hr0 >= hr1:
                            continue

                        def u(ti=ti, hr0=hr0, hr1=hr1, w0=w0, w1=w1,
                              dh=dh, dw=dw, acc3=acc3, g=g, hb=hb):
                            dst = acc3[:, hr0 - hb * 16:hr1 - hb * 16, w0:w1]
                            nc.vector.affine_then_add(
                                out=dst,
                                in0=x3[g][:, hr0 + dh:hr1 + dh, w0 + dw:w1 + dw],
                                in1=dst,
                                scale=lepec_sb[:, ti * 4 + g:ti * 4 + g + 1],
                                bias=0.0,
                            )
                        units.append(u)
                    return units

                # ---------- head: minimum to start sweep (nc2=0, g=0) ----------
                emit_qk(4, 0)
                emit_qk(0, 0)
                if _it == 0:
                    load_noncritical()

                # filler units with emission deadlines (global step index)
                fillers = [(1, lambda: emit_v(0)), (2, lambda: emit_v(1))]
                for m in range(2, NT):
                    fillers.append((m + 1, lambda m=m: emit_v(m)))
                qk_sched = [((4, 1), 3), ((5, 0), 5), ((5, 1), 6), ((1, 0), 7),
                            ((6, 0), 13), ((6, 1), 14), ((2, 0), 15),
                            ((7, 0), 21), ((7, 1), 22), ((3, 0), 23),
                            ((0, 1), 30), ((1, 1), 38), ((2, 1), 46),
                            ((3, 1), 54)]
                for (f, nc2), dl in qk_sched:
                    fillers.append((dl, lambda f=f, nc2=nc2: emit_qk(f, nc2)))
                fillers.sort(key=lambda x: x[0])
                late = [(co, 0) for co in range(CT)]  # proj nc0 halves

                # ---------- 64-step flat pipeline over sweeps (nc2, g) ----------
                sweeps = [(nc2, g) for nc2 in range(NC2) for g in range(4)]
                steps = [(nc2, g, m) for (nc2, g) in sweeps for m in range(NT)]
                state = {}

                def sweep_tiles(nc2, g):
                    out_ps = ps_small.tile([128, 512], dt.float32, tag="sm",
                                           name=f"outps{g}_{nc2}")
                    den_ps = ps_den.tile([128, 512], dt.float32, tag="den",
                                         name=f"denps{g}_{nc2}")
                    lp_ps = ps_lepe.tile([128, 512], dt.float32, tag="lp",
                                         name=f"lp{g}_{nc2}")
                    lp3 = lp_ps[:].rearrange("p (h w) -> p h w", w=Ws)
                    acc = dr_pool.tile([128, 512], dt.bfloat16, tag="dveacc",
                                       name=f"acc{g}_{nc2}")
                    nc.gpsimd.memset(acc[:], 0.0)
                    return dict(out_ps=out_ps, den_ps=den_ps, lp_ps=lp_ps,
                                acc=acc, lepe=lepe_mms(g, nc2, lp3),
                                dve=lepe_dve_units(g, nc2, acc))

                def emit_sT(nc2, g, m):
                    ncs = slice(nc2 * 512, (nc2 + 1) * 512)
                    q_t, k_t = qk_sb[g], qk_sb[4 + g]
                    sT_ps = ps_big.tile([128, 2048], dt.float32, tag="big",
                                        name=f"sT{g}_{nc2}_{m}")
                    for j in range(4):
                        nc.tensor.matmul(
                            out=sT_ps[:, j * 512:(j + 1) * 512],
                            lhsT=k_t[j * 32:(j + 1) * 32, m * 128:(m + 1) * 128],
                            rhs=q_t[j * 32:(j + 1) * 32, ncs],
                            start=True, stop=True,
                            tile_position=(j * 32, 0),
                        )
                    pT = pT_pool.tile([128, 2048], dt.bfloat16, tag="pT",
                                      name=f"pT{g}_{nc2}_{m}")
                    nc.scalar.activation(pT[:], sT_ps[:], AF.Exp, scale=SCALE)
                    if DBG and _it == 0 and g == 0 and nc2 == 0 and m == 0:
                        nc.sync.dma_start(dbg_pT[:, :], pT[:])
                    return pT

                def emit_pv(nc2, g, m, pT):
                    st = state[(nc2, g)]
                    for j in range(4):
                        h = 4 * g + j
                        nc.tensor.matmul(
                            out=st["out_ps"][j * 32:(j + 1) * 32, :],
                            lhsT=v_sb[m][:, h * 32:(h + 1) * 32],
                            rhs=pT[:, j * 512:(j + 1) * 512],
                            start=(m == 0), stop=(m == NT - 1),
                            tile_position=(0, j * 32),
                            skip_group_check=True,
                        )
                        nc.tensor.matmul(
                            out=st["den_ps"][j * 32:(j + 1) * 32, :],
                            lhsT=ones_sb[:, 0:32],
                            rhs=pT[:, j * 512:(j + 1) * 512],
                            start=(m == 0), stop=(m == NT - 1),
                            tile_position=(0, j * 32),
                            skip_group_check=True,
                        )

                def emit_epilogue(nc2, g):
                    st = state.pop((nc2, g))
                    ncs = slice(nc2 * 512, (nc2 + 1) * 512)
                    drec = dr_pool.tile([128, 512], dt.float32, tag="drec",
                                        name="drec")
                    nc.vector.reciprocal_approx_fast(out=drec[:],
                                                     in_=st["den_ps"][:])
                    tmp = dr_pool.tile([128, 512], dt.float32, tag="ntmp",
                                       name="ntmp")
                    nc.vector.tensor_mul(tmp[:], st["out_ps"][:], drec[:])
                    nc.vector.tensor_add(
                        aoT[g][:, ncs], tmp[:], aoT[g][:, ncs])
                    if DBG and _it == 0 and g == 0 and nc2 == 1:
                        nc.sync.dma_start(dbg_ao0[:, :], aoT[g][:])

                prev = None       # (nc2, g, m, pT)
                for i, (nc2, g, m) in enumerate(steps):
                    while fillers and fillers[0][0] <= i:
                        fillers.pop(0)[1]()
                    if m == 0:
                        state[(nc2, g)] = sweep_tiles(nc2, g)
                    pT = emit_sT(nc2, g, m)
                    if prev is not None:
                        pnc2, pg, pm, ppT = prev
                        emit_pv(pnc2, pg, pm, ppT)
                        if pm == NT - 1:
                            emit_epilogue(pnc2, pg)
                    st = state[(nc2, g)]
                    # hold the sweep's first LePE ops one step so they don't
                    # stall on the previous sweep's epilogue chain
                    nmm = 0 if m == 0 else (1 if m < NT - 1 else len(st["lepe"]))
                    for _ in range(min(nmm, len(st["lepe"]))):
                        st["lepe"].pop(0)()
                    ndve = 0 if m == 0 else (3 if m < NT - 1 else len(st["dve"]))
                    for _ in range(min(ndve, len(st["dve"]))):
                        st["dve"].pop(0)()
                    if m == NT - 1:
                        # pre-merge LePE (PE psum + DVE acc) into aoT now,
                        # independent of exp(m)/p@v(m); the post-p@v epilogue
                        # then only needs recip -> mul -> one add
                        ncs_s = slice(nc2 * 512, (nc2 + 1) * 512)
                        nc.vector.tensor_add(
                            aoT[g][:, ncs_s], st["lp_ps"][:], st["acc"][:])
                    if late and i >= 40 and i % 5 == 0:
                        emit_proj(*late.pop(0))
                    elif fillers and (i % 2 == 1 or i < 8):
                        fillers.pop(0)[1]()
                    prev = (nc2, g, m, pT)

                # tail
                pnc2, pg, pm, ppT = prev
                emit_pv(pnc2, pg, pm, ppT)
                emit_epilogue(pnc2, pg)
                while late:
                    emit_proj(*late.pop(0))
                for co in range(CT):
                    emit_proj(co, 1)

    nc.finalize()
    return nc


_GRAPH = None


def kernel(x, w_qkv, w_proj, b_proj, w_lepe, b_lepe, _trace=False):
    global _GRAPH, LAST_EXEC_TIME_NS, LAST_RESULTS
    from concourse.bass_utils import run_bass_kernel_spmd

    x = np.asarray(x, dtype=np.float32)
    w_qkv = np.asarray(w_qkv, dtype=np.float32)
    w_proj = np.asarray(w_proj, dtype=np.float32)
    b_proj = np.asarray(b_proj, dtype=np.float32)
    w_lepe = np.asarray(w_lepe, dtype=np.float32)
    b_lepe = np.asarray(b_lepe, dtype=np.float32)

    wqkT = np.ascontiguousarray(w_qkv[:2 * C, :].T).astype(_BF16)   # [C, 2C]
    wvT = np.ascontiguousarray(w_qkv[2 * C:, :].T).astype(_BF16)    # [C, C]
    wpT = np.ascontiguousarray(w_proj.T).astype(_BF16)              # [C, C]
    beff = (w_proj @ b_lepe + b_proj).astype(np.float32)            # [C]
    beff_t = np.ascontiguousarray(beff.reshape(4, 128).T)           # [128, 4]

    taps = [(0, 0)] + [
        (dh, dw) for dh in range(-2, 3) for dw in range(-2, 3) if (dh, dw) != (0, 0)
    ]
    wl = w_lepe.reshape(C, KS, KS)  # tap (dh,dw) -> kernel[dh+2, dw+2]
    pe_taps = [(dh, dw) for (dh, dw) in taps if abs(dh) <= 1 and abs(dw) <= 1]
    lepe_flat = np.zeros((128, 9 * 4 * 128), dtype=_BF16)
    for pi, (dh, dw) in enumerate(pe_taps):
        for g in range(4):
            col0 = (pi * 4 + g) * 128
            wcol = wl[g * 128:(g + 1) * 128, dh + 2, dw + 2].astype(_BF16)
            lepe_flat[np.arange(128), col0 + np.arange(128)] = wcol
    ones128 = np.ones((128, 32), dtype=_BF16)
    lepe_col = np.zeros((128, KS * KS * 4), dtype=np.float32)
    for ti, (dh, dw) in enumerate(taps):
        for g in range(4):
            lepe_col[:, ti * 4 + g] = wl[g * 128:(g + 1) * 128, dh + 2, dw + 2]

    in_maps = []
    for b in range(NCORES):
        xT = np.ascontiguousarray(x[b].reshape(N, C).T).astype(_BF16)  # [C, N]
        in_maps.append({
            "xT": xT,
            "wqkT": wqkT,
            "wvT": wvT,
            "wpT": wpT,
            "ones": ones128,
            "lepe": lepe_flat,
            "lepec": lepe_col,
            "beff": beff_t,
        })

    if _GRAPH is None:
        _GRAPH = _build_graph()

    res = run_bass_kernel_spmd(_GRAPH, in_maps, list(range(NCORES)), trace=_trace)
    LAST_EXEC_TIME_NS = res.exec_time_ns
    LAST_RESULTS = res

    out = np.empty((B, Hs, Ws, C), dtype=np.float32)
    for b in range(NCORES):
        yT = np.asarray(res.results[b]["out"], dtype=np.float32)  # [C, N]
        out[b] = yT.T.reshape(Hs, Ws, C)
    return out



# revision 40
# speedup vs baseline: 1.6103x; 1.6103x over previous
"""AttentionLePE distributed Trainium2 kernel (v2).

Strategy: pure data-parallel over batch (8 batch elements -> 8 NeuronCores,
no collectives). Per core, restructured vs v1:

  - p@v computed as out'[q,33] = pT_slice^T @ [v_h | 1]: a ones column
    appended to v makes the softmax denominator ride in the same matmul,
    so PE pays 33 free-rows per (head, q-chunk) instead of 2x512.
  - scores s^T = k q^T (4-head row-packed) into two 2-bank PSUM tiles
    (A: heads j0-1, B: j2-3) so exp(A) overlaps the j2/j3 matmuls and the
    B tile is consumed late in the step (no 8-bank double buffer needed).
  - exp split across engines: ScalarE runs activation(Exp) on tile A plus
    a slice of B; VectorE handles the rest of B with a one-pass Schraudolph
    fast-exp (bits = trunc(x*A+B) written through an int16 bitcast into the
    fp16 pT tile). The approximation touches only a fraction of the k-mass,
    so softmax output error stays well inside tolerance.
  - normalization: reciprocal_approx_fast on the 16 den columns, then one
    fused broadcast multiply; the normalized [q, dims] block is transposed
    back to [dims, q] on PE via identity matmuls. Six LePE taps ride the
    same PSUM accumulation group as diagonal matmuls; the evacuation add
    merges the remaining (engine-computed) LePE accumulator.
  - LePE 5x5 depthwise conv: tap (0,0) initializes the fp16 accumulator
    (VectorE tensor_scalar, 4x mode); 12 taps run as VectorE mul+add pairs,
    6 taps as GpSimd mul + GpSimd add, 6 taps on PE as above.
  - all operands fp16 (extra mantissa vs bf16); PSUM accumulation f32.

Host side pre-transposes x and weights and folds b_lepe through w_proj.
"""

import numpy as np

B, Hs, Ws, C = 8, 32, 32, 512
N = Hs * Ws          # 1024 tokens
HEADS = 16
HD = C // HEADS      # 32
KS = 5
SCALE = float(HD) ** -0.5
NCORES = 8

LAST_EXEC_TIME_NS = None
LAST_RESULTS = None

# ---- tuning knobs ----
# score strips j0/j1 exp'd on ScalarE, j2/j3 fast-exp'd on VectorE

# Schraudolph fp16 fast-exp (includes the attention scale in the multiplier)
SCH_A = SCALE * 1024.0 / float(np.log(2.0))
SCH_B = 15360.0 - 58.5

# sweep order: (nc2, g); g-priority for lepe deadlines, nc2=0 done by
# sweep index 5 so the first output half's proj starts before the tail
SWEEPS = [(0, 0), (0, 1), (1, 0), (0, 2), (1, 1), (0, 3), (1, 2), (1, 3)]

TAPS = [(0, 0)] + [
    (dh, dw) for dh in range(-2, 3) for dw in range(-2, 3) if (dh, dw) != (0, 0)
]
PE_TAPS = [(0, 1), (0, -1), (1, 0), (-1, 0), (1, 1), (-1, -1), (1, -1),
           (-1, 1), (0, 2), (0, -2), (2, 0), (-2, 0)]
RING2 = [t for t in TAPS[1:] if t not in PE_TAPS]   # 12 outer-ring taps
# per-group engine mix for the outer ring: value = number of GpSimd taps
# (rest of the 12 go to VectorE mul+add pairs).
POOL_COUNT = {0: 9, 1: 9, 2: 9, 3: 7}


def _build_graph():
    import os as _os
    LOOP = int(_os.environ.get("ATTN_LEPE_LOOP", "1"))
    DBG = _os.environ.get("ATTN_LEPE_DEBUG", "") == "1"
    ABL = _os.environ.get("ATTN_ABL", "")
    import concourse.bacc as bacc
    import concourse.mybir as mybir
    import concourse.tile as tile

    dt = mybir.dt
    AF = mybir.ActivationFunctionType
    OP = mybir.AluOpType

    nc = bacc.Bacc(
        "TRN2",
        target_bir_lowering=False,
        debug=False,
        enable_asserts=False,
        num_devices=NCORES,
    )

    xh_d = nc.dram_tensor("xh", [C, N], dt.float16, kind="ExternalInput")
    wqkT_d = nc.dram_tensor("wqkT", [C, 2 * C], dt.float16, kind="ExternalInput")
    wvT_d = nc.dram_tensor("wvT", [C, C], dt.float16, kind="ExternalInput")
    wpT_d = nc.dram_tensor("wpT", [C, C], dt.float16, kind="ExternalInput")
    ident_d = nc.dram_tensor("ident", [128, 128], dt.float32,
                             kind="ExternalInput")
    diag_d = nc.dram_tensor("diag", [128, len(PE_TAPS) * 4 * 128], dt.float16,
                            kind="ExternalInput")
    lepec_d = nc.dram_tensor("lepec", [128, 25 * 4], dt.float32,
                             kind="ExternalInput")
    beff_d = nc.dram_tensor("beff", [128, 4], dt.float32, kind="ExternalInput")
    out_d = nc.dram_tensor("out", [C, N], dt.float32, kind="ExternalOutput")
    if DBG:
        dbg_pT = [nc.dram_tensor(f"dbg_pT{m}", [128, 2048], dt.float16,
                                 kind="ExternalOutput") for m in range(8)]
        dbg_outp = nc.dram_tensor("dbg_outp", [128, 528], dt.float32,
                                  kind="ExternalOutput")
        dbg_ao = [nc.dram_tensor(f"dbg_ao{g}", [128, N], dt.float16,
                                 kind="ExternalOutput") for g in range(4)]
        dbg_qk = [nc.dram_tensor(f"dbg_qk{f}", [128, N], dt.float16,
                                 kind="ExternalOutput") for f in range(8)]
        dbg_v0 = nc.dram_tensor("dbg_v0", [128, 528], dt.float16,
                                kind="ExternalOutput")
        dbg_den = nc.dram_tensor("dbg_den", [128, 16], dt.float32,
                                 kind="ExternalOutput")
        dbg_norm = nc.dram_tensor("dbg_norm", [128, 512], dt.float32,
                                  kind="ExternalOutput")

    NT = N // 128   # 8 token tiles

    with tile.TileContext(nc) as tc:
        with (
            tc.tile_pool(name="persist", bufs=1) as persist,
            tc.tile_pool(name="pT", bufs=3) as pT_pool,
            tc.tile_pool(name="norm", bufs=5) as norm_pool,
            tc.tile_pool(name="dr", bufs=2) as dr_pool,
            tc.tile_pool(name="prd", bufs=2) as prd_pool,
            tc.tile_pool(name="prp", bufs=2) as prp_pool,
            tc.tile_pool(name="ps_strip", bufs=4, space="PSUM") as ps_strip,
            tc.tile_pool(name="ps_out", bufs=1, space="PSUM") as ps_out,
            tc.tile_pool(name="ps_small", bufs=2, space="PSUM") as ps_small,
        ):
            # ---- persistent SBUF loads (critical first) ----
            xh = []
            for g in range(4):
                t = persist.tile([128, N], dt.float16, tag=f"xh{g}",
                                 name=f"xh{g}")
                nc.sync.dma_start(t[:], xh_d[g * 128:(g + 1) * 128, :])
                xh.append(t)
            lepec_sb = persist.tile([128, 25 * 4], dt.float32, tag="lepec",
                                    name="lepec_sb")
            nc.sync.dma_start(lepec_sb[:], lepec_d[:, :])
            wqk = []
            for g in range(4):
                t = persist.tile([128, 2 * C], dt.float16, tag=f"wqk{g}",
                                 name=f"wqk{g}")
                nc.sync.dma_start(t[:], wqkT_d[g * 128:(g + 1) * 128, :])
                wqk.append(t)
            wv = []
            for g in range(4):
                t = persist.tile([128, C], dt.float16, tag=f"wv{g}",
                                 name=f"wv{g}")
                nc.sync.dma_start(t[:], wvT_d[g * 128:(g + 1) * 128, :])
                wv.append(t)
            ident_sb = persist.tile([128, 128], dt.float32, tag="ident",
                                    name="ident_sb")
            diag_sb = persist.tile([128, len(PE_TAPS) * 4 * 128], dt.float16,
                                   tag="diag", name="diag_sb")
            wp = [persist.tile([128, C], dt.float16, tag=f"wp{g}",
                               name=f"wp{g}") for g in range(4)]
            beff_sb = persist.tile([128, 4], dt.float32, tag="beff",
                                   name="beff_sb")

            def load_noncritical():
                nc.sync.dma_start(ident_sb[:], ident_d[:, :])
                nc.sync.dma_start(diag_sb[:], diag_d[:, :])
                for g in range(4):
                    nc.sync.dma_start(wp[g][:], wpT_d[g * 128:(g + 1) * 128, :])
                nc.sync.dma_start(beff_sb[:], beff_d[:, :])

            for _it in range(LOOP):
                # ---------- per-iteration tiles ----------
                qk_sb = [persist.tile([128, N], dt.float16, tag=f"qk{f}",
                                      name=f"qk{f}") for f in range(8)]
                v_sb = [persist.tile([128, 16 * 33], dt.float16, tag=f"v{m}",
                                     name=f"v{m}") for m in range(NT)]
                aoT = [persist.tile([128, N], dt.float16, tag=f"aoT{g}",
                                    name=f"aoT{g}") for g in range(4)]
                lacc = [persist.tile([128, N], dt.float16, tag=f"lacc{g}",
                                     name=f"lacc{g}") for g in range(4)]
                lacp = [persist.tile([128, N], dt.float16, tag=f"lacp{g}",
                                     name=f"lacp{g}") for g in range(4)]
                x3 = [xh[g][:].rearrange("p (h w) -> p h w", w=Ws)
                      for g in range(4)]
                l3 = [lacc[g][:].rearrange("p (h w) -> p h w", w=Ws)
                      for g in range(4)]
                lp3 = [lacp[g][:].rearrange("p (h w) -> p h w", w=Ws)
                       for g in range(4)]

                # ---------- emitters ----------
                def emit_qk(f, nc2):
                    ncs = slice(nc2 * 512, (nc2 + 1) * 512)
                    qk_ps = ps_small.tile([128, 512], dt.float32, tag="sm",
                                          name=f"qkps{f}_{nc2}")
                    for c in range(4):
                        nc.tensor.matmul(
                            out=qk_ps[:],
                            lhsT=wqk[c][:, f * 128:(f + 1) * 128],
                            rhs=xh[c][:, ncs],
                            start=(c == 0), stop=(c == 3),
                        )
                    nc.scalar.copy(qk_sb[f][:, ncs], qk_ps[:])

                def emit_v(m):
                    v_ps = ps_small.tile([128, 512], dt.float32, tag="sm",
                                         name=f"vps{m}")
                    for c in range(4):
                        nc.tensor.matmul(
                            out=v_ps[:],
                            lhsT=xh[c][:, m * 128:(m + 1) * 128],
                            rhs=wv[c][:],
                            start=(c == 0), stop=(c == 3),
                        )
                    v3 = v_sb[m][:].rearrange("p (h e) -> p h e", e=33)
                    nc.gpsimd.memset(v3[:, :, 32:33], 1.0)
                    nc.scalar.copy(
                        v3[:, :, 0:32],
                        v_ps[:].rearrange("p (h e) -> p h e", e=32))

                def emit_proj(co, nc2):
                    ncs = slice(nc2 * 512, (nc2 + 1) * 512)
                    y_ps = ps_small.tile([128, 512], dt.float32, tag="sm",
                                         name=f"yps{co}_{nc2}")
                    for c in range(4):
                        nc.tensor.matmul(
                            out=y_ps[:],
                            lhsT=wp[c][:, co * 128:(co + 1) * 128],
                            rhs=aoT[c][:, ncs],
                            start=(c == 0), stop=(c == 3),
                        )
                    y_sb = dr_pool.tile([128, 512], dt.float32, tag="ysb",
                                        name=f"ysb{co}_{nc2}")
                    nc.scalar.add(y_sb[:], y_ps[:], beff_sb[:, co:co + 1])
                    nc.sync.dma_start(out_d[co * 128:(co + 1) * 128, ncs],
                                      y_sb[:])

                # ---------- LePE engine units ----------
                def lepe_unit(ti, g, on_pool, pool_first=False):
                    dh, dw = TAPS[ti]
                    wcol = lepec_sb[:, ti * 4 + g:ti * 4 + g + 1]
                    if ti == 0:
                        nc.vector.tensor_scalar(lacc[g][:], xh[g][:], wcol,
                                                None, OP.mult)
                        return
                    r0, r1 = max(0, -dh), Hs - max(0, dh)
                    w0, w1 = max(0, -dw), Ws - max(0, dw)
                    dst = l3[g][:, r0:r1, w0:w1]
                    src = x3[g][:, r0 + dh:r1 + dh, w0 + dw:w1 + dw]
                    if pool_first:
                        # first GpSimd tap: write products over the full tile
                        # (zero-padding outside the valid window via memset)
                        nc.gpsimd.memset(lacp[g][:], 0.0)
                        nc.gpsimd.tensor_scalar(lp3[g][:, r0:r1, w0:w1], src,
                                                wcol, None, OP.mult)
                        return
                    if on_pool:
                        dstp = lp3[g][:, r0:r1, w0:w1]
                        pr = prp_pool.tile([128, Hs * Ws], dt.float16,
                                           tag="prp", name=f"prp{ti}_{g}")
                        p3 = pr[:].rearrange("p (h w) -> p h w", w=Ws)
                        pv = p3[:, r0:r1, w0:w1]
                        nc.gpsimd.tensor_scalar(pv, src, wcol, None, OP.mult)
                        nc.gpsimd.tensor_add(dstp, dstp, pv)
                    else:
                        pr = prd_pool.tile([128, Hs * Ws], dt.float16,
                                           tag="prd", name=f"prd{ti}_{g}")
                        p3 = pr[:].rearrange("p (h w) -> p h w", w=Ws)
                        pv = p3[:, r0:r1, w0:w1]
                        nc.vector.tensor_scalar(pv, src, wcol, None, OP.mult)
                        nc.vector.tensor_add(dst, dst, pv)

                # accumulator initializers first (everything else RMWs lacc)
                dve_q = [lambda g=g: lepe_unit(0, g, False) for g in range(4)]
                pool_q = []
                units_left = {g: len(RING2) for g in range(4)}

                def count_unit(fn, g):
                    def wrapped():
                        fn()
                        units_left[g] -= 1
                    return wrapped

                for g in range(4):
                    npool = POOL_COUNT[g] if ABL not in ("nolepe",) else 0
                    if ABL == "nolepe":
                        units_left[g] = 0
                        continue
                    first_pool = True
                    for k, (dh, dw) in enumerate(RING2):
                        ti = TAPS.index((dh, dw))
                        if k < len(RING2) - npool:
                            dve_q.append(count_unit(
                                lambda ti=ti, g=g: lepe_unit(ti, g, False), g))
                        else:
                            pool_q.append(count_unit(
                                lambda ti=ti, g=g, fp=first_pool:
                                lepe_unit(ti, g, True, pool_first=fp), g))
                            first_pool = False

                # ---------- sweep machinery ----------
                def emit_sT(nc2, g, m):
                    ncs = slice(nc2 * 512, (nc2 + 1) * 512)
                    k_t, q_t = qk_sb[4 + g], qk_sb[g]
                    pT = pT_pool.tile([128, 2048], dt.float16, tag="pT",
                                      name=f"pT{g}_{nc2}_{m}")
                    for j in range(2):
                        st = ps_strip.tile([128, 512], dt.float32, tag="st",
                                           name=f"st{g}_{nc2}_{m}_{j}")
                        nc.tensor.matmul(
                            out=st[:],
                            lhsT=k_t[j * 32:(j + 1) * 32,
                                     m * 128:(m + 1) * 128],
                            rhs=q_t[j * 32:(j + 1) * 32, ncs],
                            start=True, stop=True,
                            tile_position=(j * 32, 0),
                        )
                        nc.scalar.activation(pT[:, j * 512:(j + 1) * 512],
                                             st[:], AF.Exp, scale=SCALE)
                    for j in range(2, 4):
                        st = ps_strip.tile([128, 512], dt.float32, tag="st",
                                           name=f"st{g}_{nc2}_{m}_{j}")
                        nc.tensor.matmul(
                            out=st[:],
                            lhsT=k_t[j * 32:(j + 1) * 32,
                                     m * 128:(m + 1) * 128],
                            rhs=q_t[j * 32:(j + 1) * 32, ncs],
                            start=True, stop=True,
                            tile_position=(j * 32, 0),
                        )
                        nc.vector.tensor_scalar(
                            pT[:, j * 512:(j + 1) * 512].bitcast(dt.int16),
                            st[:], SCH_A, SCH_B, OP.mult, OP.add)
                    if DBG and _it == 0 and (nc2, g) == (0, 0):
                        nc.sync.dma_start(dbg_pT[m][:, :], pT[:])
                    return pT

                def emit_pv(nc2, g, m, pT, outp):
                    # matmul start=True zeroes a whole bank-aligned 2KB PSUM
                    # region: only the first matmul into each of the two
                    # bank-sized outp tiles carries start=True; the rest
                    # accumulate onto the zeroed banks.
                    outp_a, outp_b = outp
                    for qc in range(4):
                        for j in range(4):
                            h = 4 * g + j
                            dst = (outp_a[:, qc * 132 + j * 33:
                                          qc * 132 + (j + 1) * 33]
                                   if qc < 3 else
                                   outp_b[:, j * 33:(j + 1) * 33])
                            nc.tensor.matmul(
                                out=dst,
                                lhsT=pT[:, j * 512 + qc * 128:
                                        j * 512 + (qc + 1) * 128],
                                rhs=v_sb[m][:, h * 33:(h + 1) * 33],
                                start=(m == 0 and j == 0
                                       and qc in (0, 3)),
                                stop=(m == NT - 1),
                                skip_group_check=True,
                            )

                def emit_divide(nc2, g, outp):
                    outp_a, outp_b = outp
                    oa = outp_a[:, 0:396].rearrange("p (t e) -> p t e", e=33)
                    ob = outp_b[:, 0:132].rearrange("p (t e) -> p t e", e=33)
                    den16 = dr_pool.tile([128, 16], dt.float32, tag="d16",
                                         name=f"d16_{g}_{nc2}")
                    nc.vector.reciprocal_approx_fast(
                        out=den16[:, 0:12], in_=oa[:, :, 32:33].squeeze(2))
                    nc.vector.reciprocal_approx_fast(
                        out=den16[:, 12:16], in_=ob[:, :, 32:33].squeeze(2))
                    norm = norm_pool.tile([128, 512], dt.float32, tag="norm",
                                          name=f"norm{g}_{nc2}")
                    na = norm[:, 0:384].rearrange("p (t e) -> p t e", e=32)
                    nb = norm[:, 384:512].rearrange("p (t e) -> p t e", e=32)
                    da = den16[:, 0:12].unsqueeze(2).broadcast_to([128, 12, 32])
                    db = den16[:, 12:16].unsqueeze(2).broadcast_to([128, 4, 32])
                    nc.vector.tensor_tensor(na, oa[:, :, 0:32], da, OP.mult)
                    nc.vector.tensor_tensor(nb, ob[:, :, 0:32], db, OP.mult)
                    if DBG and _it == 0 and (nc2, g) == (0, 0):
                        nc.sync.dma_start(dbg_den[:, :], den16[:])
                        nc.sync.dma_start(dbg_norm[:, :], norm[:])
                    return norm

                def emit_merge(nc2, g, norm):
                    ncs = slice(nc2 * 512, (nc2 + 1) * 512)
                    tr = ps_small.tile([128, 512], dt.float32, tag="sm",
                                       name=f"tr{g}_{nc2}")
                    tr3 = tr[:].rearrange("p (r w) -> p r w", w=Ws)
                    for qc in range(4):
                        nc.tensor.matmul(
                            out=tr[:, qc * 128:(qc + 1) * 128],
                            lhsT=norm[:, qc * 128:(qc + 1) * 128],
                            rhs=ident_sb[:],
                            is_transpose=True,
                            start=(qc == 0),
                            stop=(qc == 3 and ABL == "nolepe"),
                            skip_group_check=True,
                        )
                    # LePE PE-taps accumulate into the same group
                    R0 = nc2 * 16    # image row base of this half
                    for pi, (dh, dw) in enumerate(
                            [] if ABL == "nolepe" else PE_TAPS):
                        r0, r1 = max(0, -dh), Hs - max(0, dh)
                        rr0, rr1 = max(r0, R0), min(r1, R0 + 16)
                        w0, w1 = max(0, -dw), Ws - max(0, dw)
                        last = pi == len(PE_TAPS) - 1
                        nc.tensor.matmul(
                            out=tr3[:, rr0 - R0:rr1 - R0, w0:w1],
                            lhsT=diag_sb[:, (pi * 4 + g) * 128:
                                         (pi * 4 + g + 1) * 128],
                            rhs=x3[g][:, rr0 + dh:rr1 + dh, w0 + dw:w1 + dw],
                            start=False, stop=last,
                            skip_group_check=True,
                        )
                    if ABL == "nolepe":
                        nc.vector.tensor_add(aoT[g][:, ncs], tr[:],
                                             lacc[g][:, ncs])
                    else:
                        tmp = dr_pool.tile([128, 512], dt.float16, tag="mtmp",
                                           name=f"mtmp{g}_{nc2}")
                        nc.vector.scalar_tensor_tensor(
                            tmp[:], tr[:], 1.0, lacc[g][:, ncs],
                            OP.mult, OP.add)
                        nc.vector.tensor_add(aoT[g][:, ncs], tmp[:],
                                             lacp[g][:, ncs])

                # ---------- flat schedule ----------
                steps = [(nc2, g, m) for (nc2, g) in SWEEPS for m in range(NT)]

                fillers = []
                fillers.append((0, lambda: emit_v(0)))
                fillers.append((1, lambda: emit_v(1)))
                for m in range(2, NT):
                    fillers.append((m - 1, lambda m=m: emit_v(m)))
                first_use = {}
                for i, (nc2, g) in enumerate(SWEEPS):
                    first_use.setdefault(g, i * NT)
                for g in range(4):
                    dl = max(0, first_use[g] - 7)
                    for f in (g, 4 + g):
                        for nc2 in range(2):
                            if dl == 0:
                                emit_qk(f, nc2)
                            else:
                                fillers.append((dl, lambda f=f, nc2=nc2:
                                                emit_qk(f, nc2)))
                if _it == 0:
                    load_noncritical()
                fillers.sort(key=lambda x: x[0])

                # proj of half 0 must follow the last half-0 merge
                projs = [(58 + co * 2, co, 0) for co in range(4)]
                projs += [(10**9, co, 1) for co in range(4)]
                projs.sort(key=lambda x: x[0])

                merges = []
                norms = {}
                merges_done = {0: 0, 1: 0}
                sweep_of = {}
                for si, (nc2, g) in enumerate(SWEEPS):
                    sweep_of[(nc2, g)] = si
                # merge deadline per sweep index (tail for the last two)
                MERGE_DL = {0: 30, 1: 34, 2: 36, 3: 44, 4: 46, 5: 56,
                            6: 10**9, 7: 10**9}

                outp_cur = [None]
                pend = []   # queue of (nc2, g, m, pT) awaiting pv (lag 2)

                dve_credit = [0.0]
                pool_credit = [0.0]

                def drain_pv(i):
                    if not pend:
                        return
                    if ABL == "core":
                        pend.pop(0)
                        return
                    pnc2, pg, pm, ppT = pend.pop(0)
                    if pm == 0:
                        outp_cur[0] = (
                            ps_out.tile([128, 512], dt.float32, tag="oa",
                                        name=f"oa{pg}_{pnc2}"),
                            ps_out.tile([128, 512], dt.float32, tag="ob",
                                        name=f"ob{pg}_{pnc2}"),
                        )
                    emit_pv(pnc2, pg, pm, ppT, outp_cur[0])
                    if pm == NT - 1:
                        if DBG and _it == 0 and (pnc2, pg) == (0, 0):
                            dbg_t = dr_pool.tile([128, 528], dt.float32,
                                                 tag="dbg", name="dbg_t")
                            nc.vector.tensor_copy(dbg_t[:, 0:396],
                                                  outp_cur[0][0][:, 0:396])
                            nc.vector.tensor_copy(dbg_t[:, 396:528],
                                                  outp_cur[0][1][:, 0:132])
                            nc.sync.dma_start(dbg_outp[:, :], dbg_t[:])
                        norms[(pnc2, pg)] = emit_divide(pnc2, pg, outp_cur[0])
                        si = sweep_of[(pnc2, pg)]
                        merges.append((MERGE_DL[si], pnc2, pg))
                        merges.sort(key=lambda x: x[0])

                for i, (nc2, g, m) in enumerate(steps):
                    while fillers and fillers[0][0] <= i:
                        fillers.pop(0)[1]()
                    pT = emit_sT(nc2, g, m)
                    if len(pend) >= 2:
                        drain_pv(i)
                    dve_credit[0] += 1.0 if i < 4 else 0.3
                    while dve_q and dve_credit[0] >= 1.0:
                        dve_q.pop(0)()
                        dve_credit[0] -= 1.0
                    if i >= 2:
                        pool_credit[0] += 0.85
                    while pool_q and pool_credit[0] >= 1.0:
                        pool_q.pop(0)()
                        pool_credit[0] -= 1.0
                    while (merges and merges[0][0] <= i
                           and units_left[merges[0][2]] == 0):
                        _, mnc2, mg = merges.pop(0)
                        emit_merge(mnc2, mg, norms.pop((mnc2, mg)))
                        merges_done[mnc2] += 1
                    while (projs and projs[0][0] <= i
                           and merges_done[projs[0][2]] == 4):
                        _, co, pnc2 = projs.pop(0)
                        emit_proj(co, pnc2)
                    pend.append((nc2, g, m, pT))

                # tail
                while pend:
                    drain_pv(None)
                while dve_q:
                    dve_q.pop(0)()
                while pool_q:
                    pool_q.pop(0)()
                while merges:
                    _, mnc2, mg = merges.pop(0)
                    emit_merge(mnc2, mg, norms.pop((mnc2, mg)))
                    merges_done[mnc2] += 1
                while projs:
                    _, co, pnc2 = projs.pop(0)
                    emit_proj(co, pnc2)
                if DBG and _it == 0:
                    for g in range(4):
                        nc.sync.dma_start(dbg_ao[g][:, :], aoT[g][:])
                    for f in range(8):
                        nc.sync.dma_start(dbg_qk[f][:, :], qk_sb[f][:])
                    nc.sync.dma_start(dbg_v0[:, :], v_sb[0][:])

    nc.finalize()
    return nc


_GRAPH = None


def kernel(x, w_qkv, w_proj, b_proj, w_lepe, b_lepe, _trace=False):
    global _GRAPH, LAST_EXEC_TIME_NS, LAST_RESULTS
    from concourse.bass_utils import run_bass_kernel_spmd

    x = np.asarray(x, dtype=np.float32)
    w_qkv = np.asarray(w_qkv, dtype=np.float32)
    w_proj = np.asarray(w_proj, dtype=np.float32)
    b_proj = np.asarray(b_proj, dtype=np.float32)
    w_lepe = np.asarray(w_lepe, dtype=np.float32)
    b_lepe = np.asarray(b_lepe, dtype=np.float32)

    wqkT = np.ascontiguousarray(w_qkv[:2 * C, :].T).astype(np.float16)
    wvT = np.ascontiguousarray(w_qkv[2 * C:, :].T).astype(np.float16)
    wpT = np.ascontiguousarray(w_proj.T).astype(np.float16)
    beff = (w_proj @ b_lepe + b_proj).astype(np.float32)
    beff_t = np.ascontiguousarray(beff.reshape(4, 128).T)
    ident = np.eye(128, dtype=np.float32)

    wl = w_lepe.reshape(C, KS, KS)
    lepec = np.zeros((128, 25 * 4), dtype=np.float32)
    for ti, (dh, dw) in enumerate(TAPS):
        for g in range(4):
            lepec[:, ti * 4 + g] = wl[g * 128:(g + 1) * 128, dh + 2, dw + 2]
    diag = np.zeros((128, len(PE_TAPS) * 4 * 128), dtype=np.float16)
    for pi, (dh, dw) in enumerate(PE_TAPS):
        for g in range(4):
            col0 = (pi * 4 + g) * 128
            wcol = wl[g * 128:(g + 1) * 128, dh + 2, dw + 2].astype(np.float16)
            diag[np.arange(128), col0 + np.arange(128)] = wcol

    in_maps = []
    for b in range(NCORES):
        xh = np.ascontiguousarray(x[b].reshape(N, C).T).astype(np.float16)
        in_maps.append({
            "xh": xh,
            "wqkT": wqkT,
            "wvT": wvT,
            "wpT": wpT,
            "ident": ident,
            "diag": diag,
            "lepec": lepec,
            "beff": beff_t,
        })

    if _GRAPH is None:
        _GRAPH = _build_graph()

    res = run_bass_kernel_spmd(_GRAPH, in_maps, list(range(NCORES)),
                               trace=_trace)
    LAST_EXEC_TIME_NS = res.exec_time_ns
    LAST_RESULTS = res

    out = np.empty((B, Hs, Ws, C), dtype=np.float32)
    for b in range(NCORES):
        yT = np.asarray(res.results[b]["out"], dtype=np.float32)
        out[b] = yT.T.reshape(Hs, Ws, C)
    return out
